# revision 4
# baseline (speedup 1.0000x reference)
"""Trainium2 Bass kernel for nn_MultiHeadAttention (B=2, S=2048, E=1024, H=16, D=64).

Sharding: 8 cores = 2 batches x 4 head-groups (4 heads / core, d_local=256).
Each core computes, for its (batch b, head group g):
    q = Xq[b] @ Wq[:, hs]*0.125 + bq[hs]*0.125        (transposed layout QT [256, S])
    k = Xk[b] @ Wk[:, hs] + bk[hs]                    (transposed layout KT [256, S])
    v = Xv[b] @ Wv[:, hs] + bv[hs]                    (natural layout, 65-strided + ones col)
    per head: scores^T = K_h @ Q_h^T  -> exp (ACT) -> Z|denom = expW^T.T @ [V_h|1]
    Z normalized per-partition, PE-transposed to ZT [256, S]
    partial_out = Z @ Wo[hs, :]                       ([S, E] fp32, host sums over g)
Host: transposes/casts inputs to bf16, sums the 4 partials per batch, adds bo.

Self-contained: hardcodes all shapes; requires only concourse (+ml_dtypes/numpy).
"""

import sys
import types

import numpy as np
import ml_dtypes

import concourse.bass as bass  # noqa: F401  (bass types used via tile/bacc)
import concourse.mybir as mybir
import concourse.tile as tile
from concourse import bacc
from concourse import bass_utils
from concourse.masks import make_identity

BF16 = mybir.dt.bfloat16
F32 = mybir.dt.float32
AF = mybir.ActivationFunctionType

B, S, E = 2, 2048, 1024
H, D = 16, 64
N_CORES = 8
HL = 4          # heads per core
DL = HL * D     # 256 local d
NPAIR = 2       # head pairs per core
KT_TILES = S // 128   # 16
QC = 4          # q chunks of 512
ET = E // 128   # 8 e-tiles


def _install_ntff_hook():
    """Register the axon NTFF profiling hook if the image's antenv lacks it."""
    try:
        import antenv  # noqa
        if 'antenv.axon_hooks' in sys.modules:
            return
        mod = types.ModuleType('antenv.axon_hooks')
        _hook = [None]
        mod.set_axon_ntff_profile_hook = lambda h: _hook.__setitem__(0, h)
        mod.get_axon_ntff_profile_hook = lambda: _hook[0]
        sys.modules['antenv.axon_hooks'] = mod
        setattr(antenv, 'axon_hooks', mod)
        try:
            from trn_agent_boot.trn_boot import _ntff_profile_via_ctypes
            h = _ntff_profile_via_ctypes('/opt/axon/libaxon_pjrt.so')
            if h is not None:
                mod.set_axon_ntff_profile_hook(h)
        except Exception:
            pass
    except Exception:
        pass


def build_kernel():
    nc = bacc.Bacc("TRN2", target_bir_lowering=False, debug=False,
                   enable_asserts=True, num_devices=N_CORES)

    xq_ap = nc.dram_tensor("xq_t", [E, S], BF16, kind="ExternalInput").ap()
    xk_ap = nc.dram_tensor("xk_t", [E, S], BF16, kind="ExternalInput").ap()
    xv_ap = nc.dram_tensor("xv_t", [E, S], BF16, kind="ExternalInput").ap()
    wq_ap = nc.dram_tensor("wq", [E, DL], BF16, kind="ExternalInput").ap()
    wk_ap = nc.dram_tensor("wk", [E, DL], BF16, kind="ExternalInput").ap()
    wv_ap = nc.dram_tensor("wv", [E, HL * 65], BF16, kind="ExternalInput").ap()
    bq_ap = nc.dram_tensor("bq", [DL], F32, kind="ExternalInput").ap()
    bk_ap = nc.dram_tensor("bk", [DL], F32, kind="ExternalInput").ap()
    bv_ap = nc.dram_tensor("bv", [1, HL * 65], BF16, kind="ExternalInput").ap()
    wo_ap = nc.dram_tensor("wo", [DL, E], BF16, kind="ExternalInput").ap()
    out_ap = nc.dram_tensor("out_p", [S, E], F32, kind="ExternalOutput").ap()

    from contextlib import ExitStack
    with tile.TileContext(nc) as tc, ExitStack() as ctx:
        wpool = ctx.enter_context(tc.tile_pool(name="w", bufs=1))
        xtp = ctx.enter_context(tc.tile_pool(name="xt", bufs=2))
        big = ctx.enter_context(tc.tile_pool(name="big", bufs=1))
        expp = ctx.enter_context(tc.tile_pool(name="expp", bufs=2))
        znp = ctx.enter_context(tc.tile_pool(name="znp", bufs=2))
        smal = ctx.enter_context(tc.tile_pool(name="small", bufs=2))
        stg = ctx.enter_context(tc.tile_pool(name="stg", bufs=3))
        pscore = ctx.enter_context(tc.tile_pool(name="pscore", bufs=2, space="PSUM"))
        pav = ctx.enter_context(tc.tile_pool(name="pav", bufs=1, space="PSUM"))
        ptr = ctx.enter_context(tc.tile_pool(name="ptr", bufs=1, space="PSUM"))
        ppo = ctx.enter_context(tc.tile_pool(name="ppo", bufs=2, space="PSUM"))

        # ---- persistent weights / constants ----
        wq_sb = wpool.tile([128, ET, DL], BF16, tag="wq")
        wk_sb = wpool.tile([128, ET, DL], BF16, tag="wk")
        wv_sb = wpool.tile([128, ET, HL * 65], BF16, tag="wv")
        wo_sb = wpool.tile([128, 2, E], BF16, tag="wo")
        bq_sb = wpool.tile([128, 2], F32, tag="bq")
        bk_sb = wpool.tile([128, 2], F32, tag="bk")
        bv_sb = wpool.tile([1, HL * 65], BF16, tag="bv")
        ones_col = wpool.tile([1, 128], BF16, tag="ones")
        ident = wpool.tile([128, 128], BF16, tag="ident")

        nc.sync.dma_start(wq_sb[:], wq_ap.rearrange("(eo p) d -> p eo d", p=128))
        nc.sync.dma_start(wk_sb[:], wk_ap.rearrange("(eo p) d -> p eo d", p=128))
        nc.sync.dma_start(wv_sb[:], wv_ap.rearrange("(eo p) d -> p eo d", p=128))
        nc.sync.dma_start(wo_sb[:], wo_ap.rearrange("(dt p) e -> p dt e", p=128))
        nc.sync.dma_start(bq_sb[:], bq_ap.rearrange("(h p) -> p h", p=128))
        nc.sync.dma_start(bk_sb[:], bk_ap.rearrange("(h p) -> p h", p=128))
        nc.sync.dma_start(bv_sb[:], bv_ap[:])
        nc.vector.memset(ones_col[:], 1.0)
        make_identity(nc, ident[:])

        QT = big.tile([128, NPAIR, S], BF16, tag="QT")
        KT = big.tile([128, NPAIR, S], BF16, tag="KT")
        Vones = big.tile([128, KT_TILES, HL, 65], BF16, tag="Vones")
        ZT = big.tile([128, NPAIR, S], BF16, tag="ZT")

        def load_xt(ap):
            t = xtp.tile([128, ET, S], BF16, tag="xt")
            for e in range(ET):
                nc.sync.dma_start(t[:, e, :], ap[e * 128:(e + 1) * 128, :])
            return t

        def proj_qk(dst, w_sb, b_sb, x_sb, p):
            # dst[:, p, :] (transposed proj): out[d(128), s] = W[:, p-slice].T @ X^T
            for sc in range(QC):
                ps = ppo.tile([128, 512], F32, tag="ppo")
                for e in range(ET):
                    nc.tensor.matmul(
                        ps[:], w_sb[:, e, p * 128:(p + 1) * 128],
                        x_sb[:, e, sc * 512:(sc + 1) * 512],
                        start=(e == 0), stop=(e == ET - 1))
                nc.vector.tensor_scalar_add(
                    dst[:, p, sc * 512:(sc + 1) * 512], ps[:], b_sb[:, p:p + 1])

        def proj_v(x_sb):
            # natural proj: out[s_tile(128), 4*65] = X^T.T @ Wv_aug ; +ones*bv_aug row
            for st in range(KT_TILES):
                ps = ppo.tile([128, HL * 65], F32, tag="ppo")
                for e in range(ET):
                    nc.tensor.matmul(
                        ps[:], x_sb[:, e, st * 128:(st + 1) * 128], wv_sb[:, e, :],
                        start=(e == 0), stop=False)
                nc.tensor.matmul(ps[:], ones_col[:], bv_sb[:], start=False, stop=True)
                nc.vector.tensor_copy(Vones[:, st], ps[:].rearrange("p (h d) -> p h d", h=HL))

        def scores_phase(p, qc):
            et = expp.tile([128, KT_TILES, 2, 512], BF16, tag="expT")
            for kt in range(KT_TILES):
                sc_t = pscore.tile([128, 2, 512], F32, tag="sc")
                for h in range(2):
                    nc.tensor.matmul(
                        sc_t[:, h, :],
                        KT[64 * h:64 * (h + 1), p, kt * 128:(kt + 1) * 128],
                        QT[64 * h:64 * (h + 1), p, qc * 512:(qc + 1) * 512],
                        start=True, stop=True)
                nc.scalar.activation(et[:, kt], sc_t[:], AF.Exp)
            return et

        def av_phase(p, qc, et):
            zn = znp.tile([128, 4, 2, D], BF16, tag="zn")
            for h in range(2):
                avp = pav.tile([128, 4, 65], F32, tag="av")
                for qt in range(4):
                    for kt in range(KT_TILES):
                        nc.tensor.matmul(
                            avp[:, qt, :],
                            et[:, kt, h, qt * 128:(qt + 1) * 128],
                            Vones[:, kt, 2 * p + h, :],
                            start=(kt == 0), stop=(kt == KT_TILES - 1))
                rc = smal.tile([128, 4, 1], F32, tag="rc")
                nc.vector.reciprocal(rc[:], avp[:, :, 64:65])
                nc.vector.tensor_mul(zn[:, :, h, :], avp[:, :, 0:D],
                                     rc[:].to_broadcast([128, 4, D]))
            for qt in range(4):
                tp = ptr.tile([128, 128], BF16, tag="tr")
                nc.tensor.transpose(tp[:], zn[:, qt], ident[:])
                nc.vector.tensor_copy(
                    ZT[:, p, qc * 512 + qt * 128: qc * 512 + (qt + 1) * 128], tp[:])

        def outproj():
            for st in range(KT_TILES):
                stt = stg.tile([128, 2, 512], F32, tag="stg")
                for ec in range(2):
                    ps = ppo.tile([128, 512], F32, tag="ppo")
                    for dt_ in range(2):
                        nc.tensor.matmul(
                            ps[:], ZT[:, dt_, st * 128:(st + 1) * 128],
                            wo_sb[:, dt_, ec * 512:(ec + 1) * 512],
                            start=(dt_ == 0), stop=(dt_ == 1))
                    if ec == 0:
                        nc.vector.tensor_copy(stt[:, ec], ps[:])
                    else:
                        nc.scalar.copy(stt[:, ec], ps[:])
                nc.sync.dma_start(out_ap[st * 128:(st + 1) * 128, :], stt[:])

        # ---- emission (static per-engine order ~ schedule priority) ----
        xq0 = load_xt(xq_ap)
        proj_qk(QT, wq_sb, bq_sb, xq0, 0)
        xk0 = load_xt(xk_ap)
        proj_qk(KT, wk_sb, bk_sb, xk0, 0)

        # interleave: scores one q-chunk ahead of AV to keep ACT (exp) fed
        et_prev = scores_phase(0, 0)
        xv0 = load_xt(xv_ap)
        proj_v(xv0)

        et_cur = et_prev
        et_next = scores_phase(0, 1)
        av_phase(0, 0, et_cur)
        et_cur = et_next
        et_next = scores_phase(0, 2)
        av_phase(0, 1, et_cur)
        et_cur = et_next
        et_next = scores_phase(0, 3)
        av_phase(0, 2, et_cur)
        et_cur = et_next

        xq1 = load_xt(xq_ap)
        proj_qk(QT, wq_sb, bq_sb, xq1, 1)
        xk1 = load_xt(xk_ap)
        proj_qk(KT, wk_sb, bk_sb, xk1, 1)

        et_next = scores_phase(1, 0)
        av_phase(0, 3, et_cur)
        et_cur = et_next
        et_next = scores_phase(1, 1)
        av_phase(1, 0, et_cur)
        et_cur = et_next
        et_next = scores_phase(1, 2)
        av_phase(1, 1, et_cur)
        et_cur = et_next
        et_next = scores_phase(1, 3)
        av_phase(1, 2, et_cur)
        av_phase(1, 3, et_next)

        outproj()

    nc.compile()
    return nc


def prep_inputs(query, key, value, Wq, bq, Wk, bk, Wv, bv, Wo, bo):
    """Host-side sharding: per-core input dicts (bf16, transposed/augmented)."""
    bf = ml_dtypes.bfloat16
    q32 = np.asarray(query, np.float32)
    k32 = np.asarray(key, np.float32)
    v32 = np.asarray(value, np.float32)
    Wq = np.asarray(Wq, np.float32)
    Wk = np.asarray(Wk, np.float32)
    Wv = np.asarray(Wv, np.float32)
    Wo = np.asarray(Wo, np.float32)
    bq = np.asarray(bq, np.float32)
    bk = np.asarray(bk, np.float32)
    bv = np.asarray(bv, np.float32)

    scale = 1.0 / np.sqrt(np.float32(D))
    xt = {}
    for b in range(B):
        xt[('q', b)] = np.ascontiguousarray(q32[b].T).astype(bf)
        xt[('k', b)] = np.ascontiguousarray(k32[b].T).astype(bf)
        xt[('v', b)] = np.ascontiguousarray(v32[b].T).astype(bf)

    in_maps = []
    for c in range(N_CORES):
        b, g = c // HL, c % HL
        hs = slice(g * DL, (g + 1) * DL)
        wv_aug = np.zeros((E, HL * 65), np.float32)
        bv_aug = np.zeros((1, HL * 65), np.float32)
        for h in range(HL):
            wv_aug[:, h * 65:h * 65 + D] = Wv[:, g * DL + h * D: g * DL + (h + 1) * D]
            bv_aug[0, h * 65:h * 65 + D] = bv[g * DL + h * D: g * DL + (h + 1) * D]
            bv_aug[0, h * 65 + D] = 1.0
        in_maps.append({
            "xq_t": xt[('q', b)],
            "xk_t": xt[('k', b)],
            "xv_t": xt[('v', b)],
            "wq": (Wq[:, hs] * scale).astype(bf),
            "wk": Wk[:, hs].astype(bf),
            "wv": wv_aug.astype(bf),
            "bq": (bq[hs] * scale).astype(np.float32),
            "bk": bk[hs].astype(np.float32),
            "bv": bv_aug.astype(bf),
            "wo": np.ascontiguousarray(Wo[hs, :]).astype(bf),
        })
    return in_maps


_NC_CACHE = [None]


def get_nc():
    if _NC_CACHE[0] is None:
        _install_ntff_hook()
        _NC_CACHE[0] = build_kernel()
    return _NC_CACHE[0]


def run(inputs, trace=False):
    nc = get_nc()
    in_maps = prep_inputs(**{k: v for k, v in inputs.items() if k != 'bo'},
                          bo=inputs['bo'])
    res = bass_utils.run_bass_kernel_spmd(
        nc, in_maps, core_ids=list(range(N_CORES)), trace=trace)
    bo = np.asarray(inputs['bo'], np.float32)
    out = np.empty((B, S, E), np.float32)
    for b in range(B):
        acc = np.zeros((S, E), np.float32)
        for g in range(HL):
            acc += res.results[b * HL + g]["out_p"]
        out[b] = acc + bo[None, :]
    return out, res


def kernel(**inputs):
    out, _ = run(inputs, trace=False)
    return out


# revision 13
# speedup vs baseline: 1.0600x; 1.0600x over previous
"""Trainium2 Bass kernel for nn_MultiHeadAttention (B=2, S=2048, E=1024, H=16, D=64).

Sharding: 8 cores = 2 batches x 4 head-groups (4 heads / core, d_local=256).
Each core computes, for its (batch b, head group g):
    q = Xq[b] @ Wq[:, hs]*0.125 + bq[hs]*0.125        (transposed layout QT [256, S])
    k = Xk[b] @ Wk[:, hs] + bk[hs]                    (transposed layout KT [256, S])
    v = Xv[b] @ Wv[:, hs] + bv[hs]                    (natural layout, 65-strided + ones col)
    per head: scores^T = K_h @ Q_h^T  -> exp (ACT) -> Z|denom = expW^T.T @ [V_h|1]
    Z normalized per-partition, PE-transposed to ZT [256, S]
    partial_out = Z @ Wo[hs, :]                       ([S, E] fp32, host sums over g)
Host: transposes/casts inputs to bf16, sums the 4 partials per batch, adds bo.

Self-contained: hardcodes all shapes; requires only concourse (+ml_dtypes/numpy).
"""

import sys
import types

import numpy as np
import ml_dtypes

import concourse.bass as bass  # noqa: F401  (bass types used via tile/bacc)
import concourse.mybir as mybir
import concourse.tile as tile
from concourse import bacc
from concourse import bass_utils
from concourse.masks import make_identity

BF16 = mybir.dt.bfloat16
F32 = mybir.dt.float32
AF = mybir.ActivationFunctionType

B, S, E = 2, 2048, 1024
H, D = 16, 64
N_CORES = 8
HL = 4          # heads per core
DL = HL * D     # 256 local d
NPAIR = 2       # head pairs per core
KT_TILES = S // 128   # 16
QC = 4          # q chunks of 512
ET = E // 128   # 8 e-tiles


def _install_ntff_hook():
    """Register the axon NTFF profiling hook if the image's antenv lacks it."""
    try:
        import antenv  # noqa
        if 'antenv.axon_hooks' in sys.modules:
            return
        mod = types.ModuleType('antenv.axon_hooks')
        _hook = [None]
        mod.set_axon_ntff_profile_hook = lambda h: _hook.__setitem__(0, h)
        mod.get_axon_ntff_profile_hook = lambda: _hook[0]
        sys.modules['antenv.axon_hooks'] = mod
        setattr(antenv, 'axon_hooks', mod)
        try:
            from trn_agent_boot.trn_boot import _ntff_profile_via_ctypes
            h = _ntff_profile_via_ctypes('/opt/axon/libaxon_pjrt.so')
            if h is not None:
                mod.set_axon_ntff_profile_hook(h)
        except Exception:
            pass
    except Exception:
        pass


def build_kernel():
    nc = bacc.Bacc("TRN2", target_bir_lowering=False, debug=False,
                   enable_asserts=True, num_devices=N_CORES)

    xq_ap = nc.dram_tensor("xq_t", [E, S], BF16, kind="ExternalInput").ap()
    xk_ap = nc.dram_tensor("xk_t", [E, S], BF16, kind="ExternalInput").ap()
    xv_ap = nc.dram_tensor("xv_t", [E, S], BF16, kind="ExternalInput").ap()
    wq_ap = nc.dram_tensor("wq", [E, DL], BF16, kind="ExternalInput").ap()
    wk_ap = nc.dram_tensor("wk", [E, DL], BF16, kind="ExternalInput").ap()
    wv_ap = nc.dram_tensor("wv", [E, HL * 65], BF16, kind="ExternalInput").ap()
    bq_ap = nc.dram_tensor("bq", [DL], F32, kind="ExternalInput").ap()
    bk_ap = nc.dram_tensor("bk", [DL], F32, kind="ExternalInput").ap()
    bv_ap = nc.dram_tensor("bv", [1, HL * 65], BF16, kind="ExternalInput").ap()
    wo_ap = nc.dram_tensor("wo", [DL, E], BF16, kind="ExternalInput").ap()
    out_ap = nc.dram_tensor("out_p", [S, E], F32, kind="ExternalOutput").ap()

    from contextlib import ExitStack
    with tile.TileContext(nc) as tc, ExitStack() as ctx:
        wpool = ctx.enter_context(tc.tile_pool(name="w", bufs=1))
        xtp = ctx.enter_context(tc.tile_pool(name="xt", bufs=4))
        big = ctx.enter_context(tc.tile_pool(name="big", bufs=1))
        expp = ctx.enter_context(tc.tile_pool(name="expp", bufs=3))
        znp = ctx.enter_context(tc.tile_pool(name="znp", bufs=2))
        smal = ctx.enter_context(tc.tile_pool(name="small", bufs=2))
        stg = ctx.enter_context(tc.tile_pool(name="stg", bufs=2))
        pscore = ctx.enter_context(tc.tile_pool(name="pscore", bufs=2, space="PSUM"))
        pav = ctx.enter_context(tc.tile_pool(name="pav", bufs=1, space="PSUM"))
        ptr = ctx.enter_context(tc.tile_pool(name="ptr", bufs=1, space="PSUM"))
        ppo = ctx.enter_context(tc.tile_pool(name="ppo", bufs=2, space="PSUM"))

        # ---- persistent weights / constants ----
        wq_sb = wpool.tile([128, ET, DL], BF16, tag="wq")
        wk_sb = wpool.tile([128, ET, DL], BF16, tag="wk")
        wv_sb = wpool.tile([128, ET, HL * 65], BF16, tag="wv")
        wo_sb = wpool.tile([128, 2, E], BF16, tag="wo")
        bq_sb = wpool.tile([128, 2], F32, tag="bq")
        bk_sb = wpool.tile([128, 2], F32, tag="bk")
        bv_sb = wpool.tile([1, HL * 65], BF16, tag="bv")
        ones_col = wpool.tile([1, 128], BF16, tag="ones")
        ident = wpool.tile([128, 128], BF16, tag="ident")

        nc.vector.memset(ones_col[:], 1.0)
        make_identity(nc, ident[:])

        QT = big.tile([128, NPAIR, S], BF16, tag="QT")
        KT = big.tile([128, NPAIR, S], BF16, tag="KT")
        Vones = big.tile([128, KT_TILES, HL, 65], BF16, tag="Vones")
        ZT = big.tile([128, NPAIR, S], BF16, tag="ZT")

        def load_xt_sc(ap, sc):
            # one 512-column slice of X^T: [128, 8 e-tiles, 512]
            t = xtp.tile([128, ET, 512], BF16, tag="xt")
            src = ap.rearrange("(eo p) s -> p eo s", p=128)
            nc.sync.dma_start(t[:], src[:, :, sc * 512:(sc + 1) * 512])
            return t

        def proj_qk_sc(dst, w_sb, b_sb, x_sc, p, sc):
            # dst[:, p, sc-block] (transposed proj): out[d(128), s] = W.T @ X^T
            ps = ppo.tile([128, 512], F32, tag="ppo")
            for e in range(ET):
                nc.tensor.matmul(
                    ps[:], w_sb[:, e, p * 128:(p + 1) * 128], x_sc[:, e, :],
                    start=(e == 0), stop=(e == ET - 1))
            nc.vector.tensor_scalar_add(
                dst[:, p, sc * 512:(sc + 1) * 512], ps[:], b_sb[:, p:p + 1])

        def proj_v_sc(x_sc, vsc):
            # natural proj: out[s_tile(128), 4*65] = X^T.T @ Wv_aug ; +ones*bv_aug row
            for sti in range(4):
                st = vsc * 4 + sti
                ps = ppo.tile([128, HL * 65], F32, tag="ppo")
                for e in range(ET):
                    nc.tensor.matmul(
                        ps[:], x_sc[:, e, sti * 128:(sti + 1) * 128], wv_sb[:, e, :],
                        start=(e == 0), stop=False)
                nc.tensor.matmul(ps[:], ones_col[:], bv_sb[:], start=False, stop=True)
                nc.vector.tensor_copy(Vones[:, st], ps[:].rearrange("p (h d) -> p h d", h=HL))

        def scores_kts(p, qc, et, kts):
            for kt in kts:
                sc_t = pscore.tile([128, 2, 512], F32, tag="sc")
                for h in range(2):
                    nc.tensor.matmul(
                        sc_t[:, h, :],
                        KT[64 * h:64 * (h + 1), p, kt * 128:(kt + 1) * 128],
                        QT[64 * h:64 * (h + 1), p, qc * 512:(qc + 1) * 512],
                        start=True, stop=True)
                nc.scalar.activation(et[:, kt], sc_t[:], AF.Exp)

        def scores_phase(p, qc):
            et = expp.tile([128, KT_TILES, 2, 512], BF16, tag="expT")
            scores_kts(p, qc, et, range(KT_TILES))
            return et

        def av_phase(p, qc, et):
            zn = znp.tile([128, 4, 2, D], BF16, tag="zn")
            for h in range(2):
                avp = pav.tile([128, 4, 65], F32, tag="av")
                for qt in range(4):
                    for kt in range(KT_TILES):
                        nc.tensor.matmul(
                            avp[:, qt, :],
                            et[:, kt, h, qt * 128:(qt + 1) * 128],
                            Vones[:, kt, 2 * p + h, :],
                            start=(kt == 0), stop=(kt == KT_TILES - 1))
                rc = smal.tile([128, 4, 1], F32, tag="rc")
                nc.vector.reciprocal(rc[:], avp[:, :, 64:65])
                nc.vector.tensor_mul(zn[:, :, h, :], avp[:, :, 0:D],
                                     rc[:].to_broadcast([128, 4, D]))
            for qt in range(4):
                tp = ptr.tile([128, 128], BF16, tag="tr")
                nc.tensor.transpose(tp[:], zn[:, qt], ident[:])
                nc.vector.tensor_copy(
                    ZT[:, p, qc * 512 + qt * 128: qc * 512 + (qt + 1) * 128], tp[:])

        def outproj_st(st):
            stt = stg.tile([128, 2, 512], F32, tag="stg")
            for ec in range(2):
                ps = ppo.tile([128, 512], F32, tag="ppo")
                for dt_ in range(2):
                    nc.tensor.matmul(
                        ps[:], ZT[:, dt_, st * 128:(st + 1) * 128],
                        wo_sb[:, dt_, ec * 512:(ec + 1) * 512],
                        start=(dt_ == 0), stop=(dt_ == 1))
                if ec == 0:
                    nc.vector.tensor_copy(stt[:, ec], ps[:])
                else:
                    nc.scalar.copy(stt[:, ec], ps[:])
            nc.sync.dma_start(out_ap[st * 128:(st + 1) * 128, :], stt[:])

        # ---- emission (static per-engine order ~ schedule priority) ----
        # DMA order: wq/bq, xq-sc0, wk/bk, xk-sc0 first so scores (-> exp) start ASAP
        nc.sync.dma_start(wq_sb[:], wq_ap.rearrange("(eo p) d -> p eo d", p=128))
        nc.sync.dma_start(bq_sb[:], bq_ap.rearrange("(h p) -> p h", p=128))
        nc.sync.dma_start(wk_sb[:], wk_ap.rearrange("(eo p) d -> p eo d", p=128))
        nc.sync.dma_start(bk_sb[:], bk_ap.rearrange("(h p) -> p h", p=128))

        # first q-chunk of scores interleaved with the QK projections
        et00 = expp.tile([128, KT_TILES, 2, 512], BF16, tag="expT")
        for sc in range(QC):
            xq_sc = load_xt_sc(xq_ap, sc)
            proj_qk_sc(QT, wq_sb, bq_sb, xq_sc, 0, sc)
            xk_sc = load_xt_sc(xk_ap, sc)
            proj_qk_sc(KT, wk_sb, bk_sb, xk_sc, 0, sc)
            scores_kts(0, 0, et00, range(4 * sc, 4 * sc + 4))

        # V path (needed by first av_phase)
        nc.sync.dma_start(wv_sb[:], wv_ap.rearrange("(eo p) d -> p eo d", p=128))
        nc.sync.dma_start(bv_sb[:], bv_ap[:])
        nc.sync.dma_start(wo_sb[:], wo_ap.rearrange("(dt p) e -> p dt e", p=128))
        for vsc in range(4):
            xv_sc = load_xt_sc(xv_ap, vsc)
            proj_v_sc(xv_sc, vsc)

        et_cur = et00
        et_next = scores_phase(0, 1)
        av_phase(0, 0, et_cur)
        et_cur = et_next
        et_next = scores_phase(0, 2)
        av_phase(0, 1, et_cur)
        et_cur = et_next
        et_next = scores_phase(0, 3)
        av_phase(0, 2, et_cur)
        et_cur = et_next

        for sc in range(QC):
            xq_sc = load_xt_sc(xq_ap, sc)
            proj_qk_sc(QT, wq_sb, bq_sb, xq_sc, 1, sc)
            xk_sc = load_xt_sc(xk_ap, sc)
            proj_qk_sc(KT, wk_sb, bk_sb, xk_sc, 1, sc)

        et_next = scores_phase(1, 0)
        av_phase(0, 3, et_cur)
        et_cur = et_next
        et_next = scores_phase(1, 1)
        av_phase(1, 0, et_cur)
        for st in range(0, 4):
            outproj_st(st)
        et_cur = et_next
        et_next = scores_phase(1, 2)
        av_phase(1, 1, et_cur)
        for st in range(4, 8):
            outproj_st(st)
        et_cur = et_next
        et_next = scores_phase(1, 3)
        av_phase(1, 2, et_cur)
        for st in range(8, 12):
            outproj_st(st)
        av_phase(1, 3, et_next)
        for st in range(12, 16):
            outproj_st(st)

    nc.compile()
    return nc


def prep_inputs(query, key, value, Wq, bq, Wk, bk, Wv, bv, Wo, bo):
    """Host-side sharding: per-core input dicts (bf16, transposed/augmented)."""
    bf = ml_dtypes.bfloat16
    q32 = np.asarray(query, np.float32)
    k32 = np.asarray(key, np.float32)
    v32 = np.asarray(value, np.float32)
    Wq = np.asarray(Wq, np.float32)
    Wk = np.asarray(Wk, np.float32)
    Wv = np.asarray(Wv, np.float32)
    Wo = np.asarray(Wo, np.float32)
    bq = np.asarray(bq, np.float32)
    bk = np.asarray(bk, np.float32)
    bv = np.asarray(bv, np.float32)

    scale = 1.0 / np.sqrt(np.float32(D))
    xt = {}
    for b in range(B):
        xt[('q', b)] = np.ascontiguousarray(q32[b].T).astype(bf)
        xt[('k', b)] = np.ascontiguousarray(k32[b].T).astype(bf)
        xt[('v', b)] = np.ascontiguousarray(v32[b].T).astype(bf)

    in_maps = []
    for c in range(N_CORES):
        b, g = c // HL, c % HL
        hs = slice(g * DL, (g + 1) * DL)
        wv_aug = np.zeros((E, HL * 65), np.float32)
        bv_aug = np.zeros((1, HL * 65), np.float32)
        for h in range(HL):
            wv_aug[:, h * 65:h * 65 + D] = Wv[:, g * DL + h * D: g * DL + (h + 1) * D]
            bv_aug[0, h * 65:h * 65 + D] = bv[g * DL + h * D: g * DL + (h + 1) * D]
            bv_aug[0, h * 65 + D] = 1.0
        in_maps.append({
            "xq_t": xt[('q', b)],
            "xk_t": xt[('k', b)],
            "xv_t": xt[('v', b)],
            "wq": (Wq[:, hs] * scale).astype(bf),
            "wk": Wk[:, hs].astype(bf),
            "wv": wv_aug.astype(bf),
            "bq": (bq[hs] * scale).astype(np.float32),
            "bk": bk[hs].astype(np.float32),
            "bv": bv_aug.astype(bf),
            "wo": np.ascontiguousarray(Wo[hs, :]).astype(bf),
        })
    return in_maps


_NC_CACHE = [None]


def get_nc():
    if _NC_CACHE[0] is None:
        _install_ntff_hook()
        _NC_CACHE[0] = build_kernel()
    return _NC_CACHE[0]


def run(inputs, trace=False):
    nc = get_nc()
    in_maps = prep_inputs(**{k: v for k, v in inputs.items() if k != 'bo'},
                          bo=inputs['bo'])
    res = bass_utils.run_bass_kernel_spmd(
        nc, in_maps, core_ids=list(range(N_CORES)), trace=trace)
    bo = np.asarray(inputs['bo'], np.float32)
    out = np.empty((B, S, E), np.float32)
    for b in range(B):
        acc = np.zeros((S, E), np.float32)
        for g in range(HL):
            acc += res.results[b * HL + g]["out_p"]
        out[b] = acc + bo[None, :]
    return out, res


def kernel(**inputs):
    out, _ = run(inputs, trace=False)
    return out


# revision 16
# speedup vs baseline: 1.1593x; 1.0937x over previous
"""Trainium2 Bass kernel for nn_MultiHeadAttention (B=2, S=2048, E=1024, H=16, D=64).

Sharding: 8 cores = 2 batches x 4 head-groups (4 heads / core, d_local=256).
Each core computes, for its (batch b, head group g):
    q = Xq[b] @ Wq[:, hs]*0.125 + bq[hs]*0.125        (transposed layout QT [256, S])
    k = Xk[b] @ Wk[:, hs] + bk[hs]                    (transposed layout KT [256, S])
    v = Xv[b] @ Wv[:, hs] + bv[hs]                    (natural layout, 65-strided + ones col)
    per head: scores^T = K_h @ Q_h^T  -> exp (ACT) -> Z|denom = expW^T.T @ [V_h|1]
    Z normalized per-partition, PE-transposed to ZT [256, S]
    partial_out = Z @ Wo[hs, :]                       ([S, E] fp32, host sums over g)
Host: transposes/casts inputs to bf16, sums the 4 partials per batch, adds bo.

Self-contained: hardcodes all shapes; requires only concourse (+ml_dtypes/numpy).
"""

import sys
import types

import numpy as np
import ml_dtypes

import concourse.bass as bass  # noqa: F401  (bass types used via tile/bacc)
import concourse.mybir as mybir
import concourse.tile as tile
from concourse import bacc
from concourse import bass_utils
from concourse.masks import make_identity

BF16 = mybir.dt.bfloat16
F32 = mybir.dt.float32
AF = mybir.ActivationFunctionType

B, S, E = 2, 2048, 1024
H, D = 16, 64
N_CORES = 8
HL = 4          # heads per core
DL = HL * D     # 256 local d
NPAIR = 2       # head pairs per core
KT_TILES = S // 128   # 16
QC = 4          # q chunks of 512
ET = E // 128   # 8 e-tiles


def _install_ntff_hook():
    """Register the axon NTFF profiling hook if the image's antenv lacks it."""
    try:
        import antenv  # noqa
        if 'antenv.axon_hooks' in sys.modules:
            return
        mod = types.ModuleType('antenv.axon_hooks')
        _hook = [None]
        mod.set_axon_ntff_profile_hook = lambda h: _hook.__setitem__(0, h)
        mod.get_axon_ntff_profile_hook = lambda: _hook[0]
        sys.modules['antenv.axon_hooks'] = mod
        setattr(antenv, 'axon_hooks', mod)
        try:
            from trn_agent_boot.trn_boot import _ntff_profile_via_ctypes
            h = _ntff_profile_via_ctypes('/opt/axon/libaxon_pjrt.so')
            if h is not None:
                mod.set_axon_ntff_profile_hook(h)
        except Exception:
            pass
    except Exception:
        pass


def build_kernel():
    nc = bacc.Bacc("TRN2", target_bir_lowering=False, debug=False,
                   enable_asserts=True, num_devices=N_CORES)

    xq_ap = nc.dram_tensor("xq_t", [E, S], BF16, kind="ExternalInput").ap()
    xk_ap = nc.dram_tensor("xk_t", [E, S], BF16, kind="ExternalInput").ap()
    xv_ap = nc.dram_tensor("xv_t", [E, S], BF16, kind="ExternalInput").ap()
    wq_ap = nc.dram_tensor("wq", [E, DL], BF16, kind="ExternalInput").ap()
    wk_ap = nc.dram_tensor("wk", [E, DL], BF16, kind="ExternalInput").ap()
    wv_ap = nc.dram_tensor("wv", [E, HL * 65], BF16, kind="ExternalInput").ap()
    bq_ap = nc.dram_tensor("bq", [DL], F32, kind="ExternalInput").ap()
    bk_ap = nc.dram_tensor("bk", [DL], F32, kind="ExternalInput").ap()
    bv_ap = nc.dram_tensor("bv", [1, HL * 65], BF16, kind="ExternalInput").ap()
    wo_ap = nc.dram_tensor("wo", [DL, E], BF16, kind="ExternalInput").ap()
    out_ap = nc.dram_tensor("out_p", [S, E], F32, kind="ExternalOutput").ap()

    from contextlib import ExitStack
    with tile.TileContext(nc) as tc, ExitStack() as ctx:
        wpool = ctx.enter_context(tc.tile_pool(name="w", bufs=1))
        xtp = ctx.enter_context(tc.tile_pool(name="xt", bufs=4))
        big = ctx.enter_context(tc.tile_pool(name="big", bufs=1))
        expp = ctx.enter_context(tc.tile_pool(name="expp", bufs=3))
        znp = ctx.enter_context(tc.tile_pool(name="znp", bufs=2))
        smal = ctx.enter_context(tc.tile_pool(name="small", bufs=2))
        stg = ctx.enter_context(tc.tile_pool(name="stg", bufs=2))
        pscore = ctx.enter_context(tc.tile_pool(name="pscore", bufs=2, space="PSUM"))
        pav = ctx.enter_context(tc.tile_pool(name="pav", bufs=1, space="PSUM"))
        ptr = ctx.enter_context(tc.tile_pool(name="ptr", bufs=1, space="PSUM"))
        ppo = ctx.enter_context(tc.tile_pool(name="ppo", bufs=2, space="PSUM"))

        # ---- persistent weights / constants ----
        wq_sb = wpool.tile([128, ET, DL], BF16, tag="wq")
        wk_sb = wpool.tile([128, ET, DL], BF16, tag="wk")
        wv_sb = wpool.tile([128, ET, HL * 65], BF16, tag="wv")
        wo_sb = wpool.tile([128, 2, E], BF16, tag="wo")
        bq_sb = wpool.tile([128, 2], F32, tag="bq")
        bk_sb = wpool.tile([128, 2], F32, tag="bk")
        bv_sb = wpool.tile([1, HL * 65], BF16, tag="bv")
        ones_col = wpool.tile([1, 128], BF16, tag="ones")
        ident = wpool.tile([128, 128], BF16, tag="ident")

        nc.vector.memset(ones_col[:], 1.0)
        make_identity(nc, ident[:])

        QT = big.tile([128, NPAIR, S], BF16, tag="QT")
        KT = big.tile([128, NPAIR, S], BF16, tag="KT")
        Vones = big.tile([128, KT_TILES, HL, 65], BF16, tag="Vones")
        ZT = big.tile([128, NPAIR, S], BF16, tag="ZT")

        def load_xt_sc(ap, sc):
            # one 512-column slice of X^T: [128, 8 e-tiles, 512]
            t = xtp.tile([128, ET, 512], BF16, tag="xt")
            src = ap.rearrange("(eo p) s -> p eo s", p=128)
            nc.sync.dma_start(t[:], src[:, :, sc * 512:(sc + 1) * 512])
            return t

        def proj_qk_sc(dst, w_sb, b_sb, x_sc, p, sc):
            # dst[:, p, sc-block] (transposed proj): out[d(128), s] = W.T @ X^T
            ps = ppo.tile([128, 512], F32, tag="ppo")
            for e in range(ET):
                nc.tensor.matmul(
                    ps[:], w_sb[:, e, p * 128:(p + 1) * 128], x_sc[:, e, :],
                    start=(e == 0), stop=(e == ET - 1))
            nc.vector.tensor_scalar_add(
                dst[:, p, sc * 512:(sc + 1) * 512], ps[:], b_sb[:, p:p + 1])

        def proj_v_sc(x_sc, vsc):
            # natural proj: out[s_tile(128), 4*65] = X^T.T @ Wv_aug ; +ones*bv_aug row
            for sti in range(4):
                st = vsc * 4 + sti
                ps = ppo.tile([128, HL * 65], F32, tag="ppo")
                for e in range(ET):
                    nc.tensor.matmul(
                        ps[:], x_sc[:, e, sti * 128:(sti + 1) * 128], wv_sb[:, e, :],
                        start=(e == 0), stop=False)
                nc.tensor.matmul(ps[:], ones_col[:], bv_sb[:], start=False, stop=True)
                nc.vector.tensor_copy(Vones[:, st], ps[:].rearrange("p (h d) -> p h d", h=HL))

        def scores_kts(p, qc, et, kts):
            for kt in kts:
                sc_t = pscore.tile([128, 2, 512], F32, tag="sc")
                for h in range(2):
                    nc.tensor.matmul(
                        sc_t[:, h, :],
                        KT[64 * h:64 * (h + 1), p, kt * 128:(kt + 1) * 128],
                        QT[64 * h:64 * (h + 1), p, qc * 512:(qc + 1) * 512],
                        start=True, stop=True, tile_position=(64 * h, 0))
                nc.scalar.activation(et[:, kt], sc_t[:], AF.Exp)

        def scores_phase(p, qc):
            et = expp.tile([128, KT_TILES, 2, 512], BF16, tag="expT")
            scores_kts(p, qc, et, range(KT_TILES))
            return et

        def av_phase(p, qc, et):
            zn = znp.tile([128, 4, 2, D], BF16, tag="zn")
            for h in range(2):
                avp = pav.tile([128, 4, 65], F32, tag="av")
                for qt in range(4):
                    for kt in range(KT_TILES):
                        nc.tensor.matmul(
                            avp[:, qt, :],
                            et[:, kt, h, qt * 128:(qt + 1) * 128],
                            Vones[:, kt, 2 * p + h, :],
                            start=(kt == 0), stop=(kt == KT_TILES - 1))
                rc = smal.tile([128, 4, 1], F32, tag="rc")
                nc.vector.reciprocal(rc[:], avp[:, :, 64:65])
                nc.vector.tensor_mul(zn[:, :, h, :], avp[:, :, 0:D],
                                     rc[:].to_broadcast([128, 4, D]))
            for qt in range(4):
                tp = ptr.tile([128, 128], BF16, tag="tr")
                nc.tensor.transpose(tp[:], zn[:, qt], ident[:])
                nc.vector.tensor_copy(
                    ZT[:, p, qc * 512 + qt * 128: qc * 512 + (qt + 1) * 128], tp[:])

        def outproj_st(st):
            stt = stg.tile([128, 2, 512], F32, tag="stg")
            for ec in range(2):
                ps = ppo.tile([128, 512], F32, tag="ppo")
                for dt_ in range(2):
                    nc.tensor.matmul(
                        ps[:], ZT[:, dt_, st * 128:(st + 1) * 128],
                        wo_sb[:, dt_, ec * 512:(ec + 1) * 512],
                        start=(dt_ == 0), stop=(dt_ == 1))
                nc.vector.tensor_copy(stt[:, ec], ps[:])
            nc.sync.dma_start(out_ap[st * 128:(st + 1) * 128, :], stt[:])

        # ---- emission (static per-engine order ~ schedule priority) ----
        # PE warmup (HAM): dummy matmuls on a zeroed tile during the DMA lead-in
        warm = wpool.tile([128, 512], BF16, tag="warm")
        nc.vector.memset(warm[:], 0.0)
        wps = ppo.tile([128, 512], F32, tag="ppo")
        for i in range(16):
            nc.tensor.matmul(wps[:], warm[:, 0:128], warm[:],
                             start=(i == 0), stop=(i == 15))

        # DMA order: wq, xq-sc0, wk, xk-sc0 first so scores (-> exp) start ASAP
        nc.sync.dma_start(wq_sb[:], wq_ap.rearrange("(eo p) d -> p eo d", p=128))
        nc.sync.dma_start(wk_sb[:], wk_ap.rearrange("(eo p) d -> p eo d", p=128))

        # first q-chunk of scores interleaved with the QK projections
        et00 = expp.tile([128, KT_TILES, 2, 512], BF16, tag="expT")
        for sc in range(QC):
            xq_sc = load_xt_sc(xq_ap, sc)
            xk_sc = load_xt_sc(xk_ap, sc)
            if sc == 0:
                nc.sync.dma_start(bq_sb[:], bq_ap.rearrange("(h p) -> p h", p=128))
                nc.sync.dma_start(bk_sb[:], bk_ap.rearrange("(h p) -> p h", p=128))
            proj_qk_sc(QT, wq_sb, bq_sb, xq_sc, 0, sc)
            proj_qk_sc(KT, wk_sb, bk_sb, xk_sc, 0, sc)
            scores_kts(0, 0, et00, range(4 * sc, 4 * sc + 4))

        et_cur = et00
        et_next = scores_phase(0, 1)

        # V path (needed by first av_phase) — after scores(0,1) so ACT stays fed
        nc.sync.dma_start(wv_sb[:], wv_ap.rearrange("(eo p) d -> p eo d", p=128))
        nc.sync.dma_start(bv_sb[:], bv_ap[:])
        nc.sync.dma_start(wo_sb[:], wo_ap.rearrange("(dt p) e -> p dt e", p=128))
        for vsc in range(4):
            xv_sc = load_xt_sc(xv_ap, vsc)
            proj_v_sc(xv_sc, vsc)

        av_phase(0, 0, et_cur)
        et_cur = et_next
        et_next = scores_phase(0, 2)
        av_phase(0, 1, et_cur)
        et_cur = et_next
        et_next = scores_phase(0, 3)
        av_phase(0, 2, et_cur)
        et_cur = et_next

        for sc in range(QC):
            xq_sc = load_xt_sc(xq_ap, sc)
            proj_qk_sc(QT, wq_sb, bq_sb, xq_sc, 1, sc)
            xk_sc = load_xt_sc(xk_ap, sc)
            proj_qk_sc(KT, wk_sb, bk_sb, xk_sc, 1, sc)

        et_next = scores_phase(1, 0)
        av_phase(0, 3, et_cur)
        et_cur = et_next
        et_next = scores_phase(1, 1)
        av_phase(1, 0, et_cur)
        for st in range(0, 4):
            outproj_st(st)
        et_cur = et_next
        et_next = scores_phase(1, 2)
        av_phase(1, 1, et_cur)
        for st in range(4, 8):
            outproj_st(st)
        et_cur = et_next
        et_next = scores_phase(1, 3)
        av_phase(1, 2, et_cur)
        for st in range(8, 12):
            outproj_st(st)
        av_phase(1, 3, et_next)
        for st in range(12, 16):
            outproj_st(st)

    nc.compile()
    return nc


def prep_inputs(query, key, value, Wq, bq, Wk, bk, Wv, bv, Wo, bo):
    """Host-side sharding: per-core input dicts (bf16, transposed/augmented)."""
    bf = ml_dtypes.bfloat16
    q32 = np.asarray(query, np.float32)
    k32 = np.asarray(key, np.float32)
    v32 = np.asarray(value, np.float32)
    Wq = np.asarray(Wq, np.float32)
    Wk = np.asarray(Wk, np.float32)
    Wv = np.asarray(Wv, np.float32)
    Wo = np.asarray(Wo, np.float32)
    bq = np.asarray(bq, np.float32)
    bk = np.asarray(bk, np.float32)
    bv = np.asarray(bv, np.float32)

    scale = 1.0 / np.sqrt(np.float32(D))
    xt = {}
    for b in range(B):
        xt[('q', b)] = np.ascontiguousarray(q32[b].T).astype(bf)
        xt[('k', b)] = np.ascontiguousarray(k32[b].T).astype(bf)
        xt[('v', b)] = np.ascontiguousarray(v32[b].T).astype(bf)

    in_maps = []
    for c in range(N_CORES):
        b, g = c // HL, c % HL
        hs = slice(g * DL, (g + 1) * DL)
        wv_aug = np.zeros((E, HL * 65), np.float32)
        bv_aug = np.zeros((1, HL * 65), np.float32)
        for h in range(HL):
            wv_aug[:, h * 65:h * 65 + D] = Wv[:, g * DL + h * D: g * DL + (h + 1) * D]
            bv_aug[0, h * 65:h * 65 + D] = bv[g * DL + h * D: g * DL + (h + 1) * D]
            bv_aug[0, h * 65 + D] = 1.0
        in_maps.append({
            "xq_t": xt[('q', b)],
            "xk_t": xt[('k', b)],
            "xv_t": xt[('v', b)],
            "wq": (Wq[:, hs] * scale).astype(bf),
            "wk": Wk[:, hs].astype(bf),
            "wv": wv_aug.astype(bf),
            "bq": (bq[hs] * scale).astype(np.float32),
            "bk": bk[hs].astype(np.float32),
            "bv": bv_aug.astype(bf),
            "wo": np.ascontiguousarray(Wo[hs, :]).astype(bf),
        })
    return in_maps


_NC_CACHE = [None]


def get_nc():
    if _NC_CACHE[0] is None:
        _install_ntff_hook()
        _NC_CACHE[0] = build_kernel()
    return _NC_CACHE[0]


def run(inputs, trace=False):
    nc = get_nc()
    in_maps = prep_inputs(**{k: v for k, v in inputs.items() if k != 'bo'},
                          bo=inputs['bo'])
    res = bass_utils.run_bass_kernel_spmd(
        nc, in_maps, core_ids=list(range(N_CORES)), trace=trace)
    bo = np.asarray(inputs['bo'], np.float32)
    out = np.empty((B, S, E), np.float32)
    for b in range(B):
        acc = np.zeros((S, E), np.float32)
        for g in range(HL):
            acc += res.results[b * HL + g]["out_p"]
        out[b] = acc + bo[None, :]
    return out, res


def kernel(**inputs):
    out, _ = run(inputs, trace=False)
    return out


# revision 21
# speedup vs baseline: 1.1798x; 1.0177x over previous
"""Trainium2 Bass kernel for nn_MultiHeadAttention (B=2, S=2048, E=1024, H=16, D=64).

Sharding: 8 cores = 2 batches x 4 head-groups (4 heads / core, d_local=256).
Each core computes, for its (batch b, head group g):
    q = Xq[b] @ Wq[:, hs]*0.125 + bq[hs]*0.125        (transposed layout QT [256, S])
    k = Xk[b] @ Wk[:, hs] + bk[hs]                    (transposed layout KT [256, S])
    v = Xv[b] @ Wv[:, hs] + bv[hs]                    (natural layout, 65-strided + ones col)
    per head: scores^T = K_h @ Q_h^T  -> exp (ACT) -> Z|denom = expW^T.T @ [V_h|1]
    Z normalized per-partition, PE-transposed to ZT [256, S]
    partial_out = Z @ Wo[hs, :]                       ([S, E] fp32, host sums over g)
Host: transposes/casts inputs to bf16, sums the 4 partials per batch, adds bo.

Self-contained: hardcodes all shapes; requires only concourse (+ml_dtypes/numpy).
"""

import sys
import types

import numpy as np
import ml_dtypes

import concourse.bass as bass  # noqa: F401  (bass types used via tile/bacc)
import concourse.mybir as mybir
import concourse.tile as tile
from concourse import bacc
from concourse import bass_utils
from concourse.masks import make_identity

BF16 = mybir.dt.bfloat16
F32 = mybir.dt.float32
AF = mybir.ActivationFunctionType

B, S, E = 2, 2048, 1024
H, D = 16, 64
N_CORES = 8
HL = 4          # heads per core
DL = HL * D     # 256 local d
NPAIR = 2       # head pairs per core
KT_TILES = S // 128   # 16
QC = 4          # q chunks of 512
ET = E // 128   # 8 e-tiles


def _install_ntff_hook():
    """Register the axon NTFF profiling hook if the image's antenv lacks it."""
    try:
        import antenv  # noqa
        if 'antenv.axon_hooks' in sys.modules:
            return
        mod = types.ModuleType('antenv.axon_hooks')
        _hook = [None]
        mod.set_axon_ntff_profile_hook = lambda h: _hook.__setitem__(0, h)
        mod.get_axon_ntff_profile_hook = lambda: _hook[0]
        sys.modules['antenv.axon_hooks'] = mod
        setattr(antenv, 'axon_hooks', mod)
        try:
            from trn_agent_boot.trn_boot import _ntff_profile_via_ctypes
            h = _ntff_profile_via_ctypes('/opt/axon/libaxon_pjrt.so')
            if h is not None:
                mod.set_axon_ntff_profile_hook(h)
        except Exception:
            pass
    except Exception:
        pass


def build_kernel():
    nc = bacc.Bacc("TRN2", target_bir_lowering=False, debug=False,
                   enable_asserts=True, num_devices=N_CORES)

    # all inputs pre-arranged on host to be contiguous for their SBUF tiles
    xq_ap = nc.dram_tensor("xq_t", [QC, 128, ET, 512], BF16, kind="ExternalInput").ap()
    xk_ap = nc.dram_tensor("xk_t", [QC, 128, ET, 512], BF16, kind="ExternalInput").ap()
    xv_ap = nc.dram_tensor("xv_t", [QC, 128, ET, 512], BF16, kind="ExternalInput").ap()
    wq_ap = nc.dram_tensor("wq", [128, ET, DL], BF16, kind="ExternalInput").ap()
    wk_ap = nc.dram_tensor("wk", [128, ET, DL], BF16, kind="ExternalInput").ap()
    wv_ap = nc.dram_tensor("wv", [128, ET, HL * 65], BF16, kind="ExternalInput").ap()
    bq_ap = nc.dram_tensor("bq", [128, 2], F32, kind="ExternalInput").ap()
    bk_ap = nc.dram_tensor("bk", [128, 2], F32, kind="ExternalInput").ap()
    bv_ap = nc.dram_tensor("bv", [1, HL * 65], BF16, kind="ExternalInput").ap()
    wo_ap = nc.dram_tensor("wo", [128, 2, E], BF16, kind="ExternalInput").ap()
    out_ap = nc.dram_tensor("out_p", [S, E], F32, kind="ExternalOutput").ap()

    from contextlib import ExitStack
    with tile.TileContext(nc) as tc, ExitStack() as ctx:
        wpool = ctx.enter_context(tc.tile_pool(name="w", bufs=1))
        xtp = ctx.enter_context(tc.tile_pool(name="xt", bufs=4))
        big = ctx.enter_context(tc.tile_pool(name="big", bufs=1))
        expp = ctx.enter_context(tc.tile_pool(name="expp", bufs=3))
        znp = ctx.enter_context(tc.tile_pool(name="znp", bufs=2))
        smal = ctx.enter_context(tc.tile_pool(name="small", bufs=2))
        stg = ctx.enter_context(tc.tile_pool(name="stg", bufs=2))
        pscore = ctx.enter_context(tc.tile_pool(name="pscore", bufs=2, space="PSUM"))
        pav = ctx.enter_context(tc.tile_pool(name="pav", bufs=1, space="PSUM"))
        ptr = ctx.enter_context(tc.tile_pool(name="ptr", bufs=1, space="PSUM"))
        ppo = ctx.enter_context(tc.tile_pool(name="ppo", bufs=2, space="PSUM"))

        # ---- persistent weights / constants ----
        wq_sb = wpool.tile([128, ET, DL], BF16, tag="wq")
        wk_sb = wpool.tile([128, ET, DL], BF16, tag="wk")
        wv_sb = wpool.tile([128, ET, HL * 65], BF16, tag="wv")
        wo_sb = wpool.tile([128, 2, E], BF16, tag="wo")
        bq_sb = wpool.tile([128, 2], F32, tag="bq")
        bk_sb = wpool.tile([128, 2], F32, tag="bk")
        bv_sb = wpool.tile([1, HL * 65], BF16, tag="bv")
        ones_col = wpool.tile([1, 128], BF16, tag="ones")
        ident = wpool.tile([128, 128], BF16, tag="ident")

        nc.vector.memset(ones_col[:], 1.0)
        make_identity(nc, ident[:])

        QT = big.tile([128, NPAIR, S], BF16, tag="QT")
        KT = big.tile([128, NPAIR, S], BF16, tag="KT")
        Vones = big.tile([128, KT_TILES, HL, 65], BF16, tag="Vones")
        ZT = big.tile([128, NPAIR, S], BF16, tag="ZT")

        def load_xt_sc(ap, sc):
            # one 512-column slice of X^T: [128, 8 e-tiles, 512], contiguous in DRAM
            t = xtp.tile([128, ET, 512], BF16, tag="xt")
            nc.sync.dma_start(t[:], ap[sc])
            return t

        def proj_qk_sc(dst, w_sb, b_sb, x_sc, p, sc):
            # dst[:, p, sc-block] (transposed proj): out[d(128), s] = W.T @ X^T
            ps = ppo.tile([128, 512], F32, tag="ppo")
            for e in range(ET):
                nc.tensor.matmul(
                    ps[:], w_sb[:, e, p * 128:(p + 1) * 128], x_sc[:, e, :],
                    start=(e == 0), stop=(e == ET - 1))
            nc.vector.tensor_scalar_add(
                dst[:, p, sc * 512:(sc + 1) * 512], ps[:], b_sb[:, p:p + 1])

        def proj_v_sc(x_sc, vsc):
            # natural proj: out[s_tile(128), 4*65] = X^T.T @ Wv_aug ; +ones*bv_aug row
            for sti in range(4):
                st = vsc * 4 + sti
                ps = ppo.tile([128, HL * 65], F32, tag="ppo")
                for e in range(ET):
                    nc.tensor.matmul(
                        ps[:], x_sc[:, e, sti * 128:(sti + 1) * 128], wv_sb[:, e, :],
                        start=(e == 0), stop=False)
                nc.tensor.matmul(ps[:], ones_col[:], bv_sb[:], start=False, stop=True)
                nc.vector.tensor_copy(Vones[:, st], ps[:].rearrange("p (h d) -> p h d", h=HL))

        def scores_kts(p, qc, et, kts):
            for kt in kts:
                sc_t = pscore.tile([128, 2, 512], F32, tag="sc")
                for h in range(2):
                    nc.tensor.matmul(
                        sc_t[:, h, :],
                        KT[64 * h:64 * (h + 1), p, kt * 128:(kt + 1) * 128],
                        QT[64 * h:64 * (h + 1), p, qc * 512:(qc + 1) * 512],
                        start=True, stop=True, tile_position=(64 * h, 0))
                nc.scalar.activation(et[:, kt], sc_t[:], AF.Exp)

        def scores_phase(p, qc):
            et = expp.tile([128, KT_TILES, 2, 512], BF16, tag="expT")
            scores_kts(p, qc, et, range(KT_TILES))
            return et

        def av_phase(p, qc, et):
            zn = znp.tile([128, 4, 2, D], BF16, tag="zn")
            for h in range(2):
                avp = pav.tile([128, 4, 65], F32, tag="av")
                for qt in range(4):
                    for kt in range(KT_TILES):
                        nc.tensor.matmul(
                            avp[:, qt, :],
                            et[:, kt, h, qt * 128:(qt + 1) * 128],
                            Vones[:, kt, 2 * p + h, :],
                            start=(kt == 0), stop=(kt == KT_TILES - 1))
                rc = smal.tile([128, 4, 1], F32, tag="rc")
                nc.vector.reciprocal(rc[:], avp[:, :, 64:65])
                nc.vector.tensor_mul(zn[:, :, h, :], avp[:, :, 0:D],
                                     rc[:].to_broadcast([128, 4, D]))
            for qt in range(4):
                tp = ptr.tile([128, 128], BF16, tag="tr")
                nc.tensor.transpose(tp[:], zn[:, qt], ident[:])
                nc.vector.tensor_copy(
                    ZT[:, p, qc * 512 + qt * 128: qc * 512 + (qt + 1) * 128], tp[:])

        def outproj_st(st):
            stt = stg.tile([128, 2, 512], F32, tag="stg")
            for ec in range(2):
                ps = ppo.tile([128, 512], F32, tag="ppo")
                for dt_ in range(2):
                    nc.tensor.matmul(
                        ps[:], ZT[:, dt_, st * 128:(st + 1) * 128],
                        wo_sb[:, dt_, ec * 512:(ec + 1) * 512],
                        start=(dt_ == 0), stop=(dt_ == 1))
                nc.vector.tensor_copy(stt[:, ec], ps[:])
            nc.sync.dma_start(out_ap[st * 128:(st + 1) * 128, :], stt[:])

        # ---- emission (static per-engine order ~ schedule priority) ----
        # PE warmup (HAM): dummy matmuls on a zeroed tile during the DMA lead-in
        warm = wpool.tile([128, 512], BF16, tag="warm")
        nc.vector.memset(warm[:], 0.0)
        wps = ppo.tile([128, 512], F32, tag="ppo")
        for i in range(16):
            nc.tensor.matmul(wps[:], warm[:, 0:128], warm[:],
                             start=(i == 0), stop=(i == 15))

        # DMA order: xq-sc0, wq, xk-sc0, wk first so scores (-> exp) start ASAP
        nc.sync.dma_start(wq_sb[:], wq_ap[:])
        nc.sync.dma_start(wk_sb[:], wk_ap[:])

        # first q-chunk of scores interleaved with the QK projections
        et00 = expp.tile([128, KT_TILES, 2, 512], BF16, tag="expT")
        for sc in range(QC):
            xq_sc = load_xt_sc(xq_ap, sc)
            xk_sc = load_xt_sc(xk_ap, sc)
            if sc == 0:
                nc.sync.dma_start(bq_sb[:], bq_ap[:])
                nc.sync.dma_start(bk_sb[:], bk_ap[:])
            proj_qk_sc(QT, wq_sb, bq_sb, xq_sc, 0, sc)
            proj_qk_sc(KT, wk_sb, bk_sb, xk_sc, 0, sc)
            scores_kts(0, 0, et00, range(4 * sc, 4 * sc + 4))

        # V path loads (needed by first av_phase)
        nc.sync.dma_start(wv_sb[:], wv_ap[:])
        nc.sync.dma_start(bv_sb[:], bv_ap[:])
        nc.sync.dma_start(wo_sb[:], wo_ap[:])

        # scores(0,1) with V-projection interleaved so ACT stays fed and PE
        # spreads the V work through the exp(0,0) window
        et_cur = et00
        et_next = expp.tile([128, KT_TILES, 2, 512], BF16, tag="expT")
        for vsc in range(4):
            scores_kts(0, 1, et_next, range(4 * vsc, 4 * vsc + 4))
            xv_sc = load_xt_sc(xv_ap, vsc)
            proj_v_sc(xv_sc, vsc)

        av_phase(0, 0, et_cur)
        et_cur = et_next
        et_next = scores_phase(0, 2)
        av_phase(0, 1, et_cur)
        et_cur = et_next
        et_next = scores_phase(0, 3)
        av_phase(0, 2, et_cur)
        et_cur = et_next

        for sc in range(QC):
            xq_sc = load_xt_sc(xq_ap, sc)
            proj_qk_sc(QT, wq_sb, bq_sb, xq_sc, 1, sc)
            xk_sc = load_xt_sc(xk_ap, sc)
            proj_qk_sc(KT, wk_sb, bk_sb, xk_sc, 1, sc)

        et_next = scores_phase(1, 0)
        av_phase(0, 3, et_cur)
        et_cur = et_next
        et_next = scores_phase(1, 1)
        av_phase(1, 0, et_cur)
        for st in range(0, 4):
            outproj_st(st)
        et_cur = et_next
        et_next = scores_phase(1, 2)
        av_phase(1, 1, et_cur)
        for st in range(4, 8):
            outproj_st(st)
        et_cur = et_next
        et_next = scores_phase(1, 3)
        av_phase(1, 2, et_cur)
        for st in range(8, 12):
            outproj_st(st)
        av_phase(1, 3, et_next)
        for st in range(12, 16):
            outproj_st(st)

    nc.compile()
    return nc


def prep_inputs(query, key, value, Wq, bq, Wk, bk, Wv, bv, Wo, bo):
    """Host-side sharding: per-core input dicts (bf16, transposed/augmented)."""
    bf = ml_dtypes.bfloat16
    q32 = np.asarray(query, np.float32)
    k32 = np.asarray(key, np.float32)
    v32 = np.asarray(value, np.float32)
    Wq = np.asarray(Wq, np.float32)
    Wk = np.asarray(Wk, np.float32)
    Wv = np.asarray(Wv, np.float32)
    Wo = np.asarray(Wo, np.float32)
    bq = np.asarray(bq, np.float32)
    bk = np.asarray(bk, np.float32)
    bv = np.asarray(bv, np.float32)

    scale = 1.0 / np.sqrt(np.float32(D))

    def xt_layout(x2d):
        # [S, E] -> X^T [E, S] -> [sc, p, eo, j] contiguous tile layout
        a = x2d.T.reshape(ET, 128, QC, 512).transpose(2, 1, 0, 3)
        return np.ascontiguousarray(a).astype(bf)

    def w_layout(w2d):
        # [E, D'] -> [p, eo, D'] contiguous
        a = w2d.reshape(ET, 128, w2d.shape[1]).transpose(1, 0, 2)
        return np.ascontiguousarray(a).astype(bf)

    xt = {}
    for b in range(B):
        xt[('q', b)] = xt_layout(q32[b])
        xt[('k', b)] = xt_layout(k32[b])
        xt[('v', b)] = xt_layout(v32[b])

    in_maps = []
    for c in range(N_CORES):
        b, g = c // HL, c % HL
        hs = slice(g * DL, (g + 1) * DL)
        wv_aug = np.zeros((E, HL * 65), np.float32)
        bv_aug = np.zeros((1, HL * 65), np.float32)
        for h in range(HL):
            wv_aug[:, h * 65:h * 65 + D] = Wv[:, g * DL + h * D: g * DL + (h + 1) * D]
            bv_aug[0, h * 65:h * 65 + D] = bv[g * DL + h * D: g * DL + (h + 1) * D]
            bv_aug[0, h * 65 + D] = 1.0
        in_maps.append({
            "xq_t": xt[('q', b)],
            "xk_t": xt[('k', b)],
            "xv_t": xt[('v', b)],
            "wq": w_layout(Wq[:, hs] * scale),
            "wk": w_layout(Wk[:, hs]),
            "wv": w_layout(wv_aug),
            "bq": np.ascontiguousarray(
                (bq[hs] * scale).reshape(2, 128).T).astype(np.float32),
            "bk": np.ascontiguousarray(
                bk[hs].reshape(2, 128).T).astype(np.float32),
            "bv": bv_aug.astype(bf),
            "wo": np.ascontiguousarray(
                Wo[hs, :].reshape(2, 128, E).transpose(1, 0, 2)).astype(bf),
        })
    return in_maps


_NC_CACHE = [None]


def get_nc():
    if _NC_CACHE[0] is None:
        _install_ntff_hook()
        _NC_CACHE[0] = build_kernel()
    return _NC_CACHE[0]


def run(inputs, trace=False):
    nc = get_nc()
    in_maps = prep_inputs(**{k: v for k, v in inputs.items() if k != 'bo'},
                          bo=inputs['bo'])
    res = bass_utils.run_bass_kernel_spmd(
        nc, in_maps, core_ids=list(range(N_CORES)), trace=trace)
    bo = np.asarray(inputs['bo'], np.float32)
    out = np.empty((B, S, E), np.float32)
    for b in range(B):
        acc = np.zeros((S, E), np.float32)
        for g in range(HL):
            acc += res.results[b * HL + g]["out_p"]
        out[b] = acc + bo[None, :]
    return out, res


def kernel(**inputs):
    out, _ = run(inputs, trace=False)
    return out
